# revision 13
# baseline (speedup 1.0000x reference)
"""Trainium2 Bass kernel for MatchingLayer (cosine-sim + per-row top-K mean).

Computation (reference):
  mask[m]  = all(query_label[m] == color)            # per-COLUMN property
  sim      = l2norm_rows(s) @ l2norm_rows(q).T       # [N=9216, M=9216], C=256
  fg_score = mean(top20(sim over fg columns)) per row -> (96, 96)
  bg_score = mean(top20(sim over bg columns)) per row -> (96, 96)

Sharding: rows split across 8 cores, 1152 rows each. Q replicated,
reordered fg-first; both s and q l2-normalized + bf16 on host.

Trace-driven evolution of the 137-163us baseline:
 * Slab input layout [128, 2*cols] (contraction-chunk concat per
   partition): long contiguous DMA lines instead of the 3168 small
   descriptors that trickled input in until the final microseconds and
   paced the whole kernel.
 * fg scoring via threshold-sum: exact top-8 per 192-wide span (6 max8),
   cascade -> exact top-24 of candidates, tau = 20th largest, then ACT
   computes sum(relu(x - tau)) over the fg columns in one instruction;
   fg = tau + S/20. Exact when the candidates cover the top-20,
   second-order-small error otherwise (sim/HW: 3.7e-3 relmax).
 * The next block's fg-tile matmuls are emitted after the 5th bg tile:
   by then this block's fg relu has released the fg banks (fgp bufs=1),
   so the PE fills the next fg tile mid-block and the DVE does not wait
   at the block boundary (~1.3us/block gap in the un-pipelined trace).
   Emitting them earlier (bufs=2 fgA variant) made every PSUM scan
   ~100ns slower from PE-write/DVE-read PSUM contention -- a net loss.
 * bg unchanged: exact top-8 per 1024 PSUM tile + cascade top-24 +
   mean(top20) via ACT accum (4.5e-3).

Per 128-row block: matmul 512-wide bf16 pieces into fg tile (3 banks,
bufs=1) + rolling 1024-col bg tiles (2x2 banks) + 512 tail bank. Every
sim value crosses DVE max8 at ~1 elem/cycle -- the architectural floor
(ACT cannot max, GPSIMD cannot read PSUM, matmul only writes PSUM).
"""

import sys

sys.path.insert(0, "/opt/trn_rl_repo")

import numpy as np

C = 256
H = W = 96
N = H * W            # 9216 support locations (rows of sim)
M = H * W            # 9216 query locations  (cols of sim)
NCORES = 8
R = N // NCORES      # 1152 rows per core
RB = R // 128        # 9 row blocks per core
K = 20
NEG = -1.0e30
FGW = 192            # fg candidate span width

_CACHE = {}


def _build_program(Mf):
    import concourse.mybir as mybir
    from concourse import bacc, tile

    f32 = mybir.dt.float32
    bf16 = mybir.dt.bfloat16

    nc = bacc.Bacc()
    s16_in = nc.declare_dram_parameter("s16", [128, 2 * R], bf16, isOutput=False)
    q_in = nc.declare_dram_parameter("q", [128, 2 * M], bf16, isOutput=False)
    fg_out = nc.declare_dram_parameter("fg", [128, RB], f32, isOutput=True)
    bg_out = nc.declare_dram_parameter("bg", [128, RB], f32, isOutput=True)

    assert 1024 < Mf <= 1528, f"unexpected fg column count {Mf}"

    # column tiling: fg tile [0,1536) = fg Mf + bg head;
    # then 7 bg tiles of 1024: [1536, 8704); tail [8704, 9216).
    BG1 = 1536
    NBG = 7
    TAIL = 8704
    assert BG1 + NBG * 1024 == TAIL and TAIL + 512 == M

    FG_SPANS = []
    b = 0
    while b < Mf:
        e = min(b + FGW, Mf)
        if 0 < Mf - e < 8:
            e = Mf
        FG_SPANS.append((b, e))
        b = e
    NFG = len(FG_SPANS)

    NBGL = 2 + NBG  # number of 8-wide bg candidate lists

    with tile.TileContext(nc) as tc:
        with (
            tc.tile_pool(name="const", bufs=1) as cp,
            tc.tile_pool(name="work", bufs=2) as wp,
            tc.tile_pool(name="fgp", bufs=1, space="PSUM") as fgp,
            tc.tile_pool(name="bgp", bufs=2, space="PSUM") as bgp,
            tc.tile_pool(name="tlp", bufs=1, space="PSUM") as tlp,
        ):
            Qs = cp.tile([128, 2 * M], bf16, tag="qs", name="qs")
            Ss = cp.tile([128, 2 * R], bf16, tag="ss", name="ss")
            out_fg = cp.tile([128, RB], f32, tag="out_fg")
            out_bg = cp.tile([128, RB], f32, tag="out_bg")

            # --- input DMAs: long contiguous per-partition lines ---
            nc.sync.dma_start(out=Ss[:], in_=s16_in[:])
            qranges = [(0, 1536), (1536, 5120), (5120, 9216)]
            for lo, hi in qranges:
                for kc in range(2):
                    nc.sync.dma_start(
                        out=Qs[:, kc * M + lo:kc * M + hi],
                        in_=q_in[:, kc * M + lo:kc * M + hi],
                    )

            MMW = 512  # matmul moving width (one PSUM bank per MM output;
                       # 1024-wide outputs fail neuronxcc codegen)

            def mm_fill(pt, psl, rb, csl):
                """Accumulate sim rows rb*128.. into pt[:, psl] for cols csl."""
                rlo = rb * 128
                width = psl.stop - psl.start
                assert width == csl.stop - csl.start
                pieces = []
                b = 0
                while b < width:
                    e = min(b + MMW, width)
                    pieces.append((b, e))
                    b = e
                for kc in range(2):
                    st = kc * R + rlo
                    qt = kc * M + csl.start
                    for b, e in pieces:
                        nc.tensor.matmul(
                            pt[:, psl.start + b:psl.start + e],
                            Ss[:, st:st + 128],
                            Qs[:, qt + b:qt + e],
                            start=(kc == 0), stop=(kc == 1),
                        )

            def emit_fg_tile(rb):
                t = fgp.tile([128, 1536], f32, tag="fgt")
                mm_fill(t, slice(0, 1024), rb, slice(0, 1024))
                mm_fill(t, slice(1024, 1536), rb, slice(1024, 1536))
                return t

            fgt_next = emit_fg_tile(0)

            # --- main loop: 9 row blocks ---
            for rb in range(RB):
                fgl = wp.tile([128, NFG * 8], f32, tag="fgl")
                fglb = wp.tile([128, NFG * 8], f32, tag="fglb")
                bgl = wp.tile([128, NBGL * 8], f32, tag="bgl")
                bglb = wp.tile([128, NBGL * 8], f32, tag="bglb")
                gf = wp.tile([128, 24], f32, tag="gf")
                gb = wp.tile([128, 24], f32, tag="gb")
                ntau = wp.tile([128, 1], f32, tag="ntau")
                facc = wp.tile([128, 1], f32, tag="facc")
                scr = wp.tile([128, K], f32, tag="scr")
                # relu spill target in SBUF: keeps ACT write traffic off
                # PSUM, which the DVE is scanning concurrently
                rscr = wp.tile([128, 1536], bf16, tag="rscr")

                # ---- fg tile scans ----
                fgt = fgt_next
                for i, (lo, hi) in enumerate(FG_SPANS):
                    nc.vector.max(fgl[:, i * 8:(i + 1) * 8], fgt[:, lo:hi])
                # bg span inside fg tile (cols Mf..1536)
                nc.vector.max(bgl[:, 0:8], fgt[:, Mf:1536])

                # ---- fg cascade -> exact top-24 of candidates ----
                nc.vector.max(gf[:, 0:8], fgl[:])
                nc.vector.match_replace(fglb[:], gf[:, 0:8], fgl[:], NEG)
                nc.vector.max(gf[:, 8:16], fglb[:])
                nc.vector.match_replace(fgl[:], gf[:, 8:16], fglb[:], NEG)
                nc.vector.max(gf[:, 16:24], fgl[:])
                # tau = 20th largest candidate; fg = tau + relu-sum/K
                nc.vector.tensor_scalar_mul(ntau[:], gf[:, 19:20], -1.0)
                nc.scalar.activation(
                    out=rscr[:, 0:Mf], in_=fgt[:, 0:Mf],
                    func=mybir.ActivationFunctionType.Relu,
                    bias=ntau[:, 0:1], scale=1.0,
                    accum_out=facc[:])
                nc.vector.tensor_scalar(
                    out=out_fg[:, rb:rb + 1], in0=facc[:],
                    scalar1=1.0 / K, scalar2=gf[:, 19:20],
                    op0=mybir.AluOpType.mult, op1=mybir.AluOpType.add)

                # ---- bg sections ----
                # tail MMs first: its bank is free from the previous block,
                # giving the PE runway before the bgp bufs=2 stall
                tlt = tlp.tile([128, 512], f32, tag="tlt")
                mm_fill(tlt, slice(0, 512), rb, slice(TAIL, M))
                for j in range(NBG):
                    lo = BG1 + 1024 * j
                    bgt = bgp.tile([128, 1024], f32, tag="bgt")
                    mm_fill(bgt, slice(0, 1024), rb, slice(lo, lo + 1024))
                    nc.vector.max(bgl[:, (1 + j) * 8:(2 + j) * 8], bgt[:])
                    if j == 4 and rb + 1 < RB:
                        # next block's fg tile: by now this block's fg relu
                        # has released the fgp banks, and the PE picks these
                        # up mid-block instead of at the boundary
                        fgt_next = emit_fg_tile(rb + 1)
                nc.vector.max(bgl[:, (1 + NBG) * 8:(2 + NBG) * 8], tlt[:])

                # ---- bg cascade -> exact top-24 + mean(top20) ----
                nc.vector.max(gb[:, 0:8], bgl[:])
                nc.vector.match_replace(bglb[:], gb[:, 0:8], bgl[:], NEG)
                nc.vector.max(gb[:, 8:16], bglb[:])
                nc.vector.match_replace(bgl[:], gb[:, 8:16], bglb[:], NEG)
                nc.vector.max(gb[:, 16:24], bgl[:])
                nc.scalar.activation(
                    out=scr[:, 0:K], in_=gb[:, 0:K],
                    func=mybir.ActivationFunctionType.Copy,
                    scale=1.0 / K, accum_out=out_bg[:, rb:rb + 1])

            nc.sync.dma_start(out=fg_out[:], in_=out_fg[:])
            nc.sync.dma_start(out=bg_out[:], in_=out_bg[:])

    nc.compile()
    return nc


def _bf16(a):
    import ml_dtypes
    return np.ascontiguousarray(a.astype(ml_dtypes.bfloat16))


def _prep_inputs(query_label, color, q_feat, s_feat):
    mask = np.all(np.asarray(query_label) == np.asarray(color), axis=-1).reshape(-1)
    Mf = int(mask.sum())
    q = np.asarray(q_feat, dtype=np.float32)[0].reshape(C, M)  # [C, M]
    s = np.asarray(s_feat, dtype=np.float32)[0].reshape(C, N)
    qn = q / np.maximum(np.sqrt(np.sum(q * q, axis=0)), np.float32(1e-12))[None, :]
    sn = s / np.maximum(np.sqrt(np.sum(s * s, axis=0)), np.float32(1e-12))[None, :]
    order = np.concatenate([np.nonzero(mask)[0], np.nonzero(~mask)[0]])
    Qn = np.ascontiguousarray(qn[:, order], dtype=np.float32)
    return Mf, Qn, sn


def _run(query_label, color, q_feat, s_feat, trace=False):
    from concourse.bass_utils import run_bass_kernel_spmd

    Mf, Qn, sn = _prep_inputs(query_label, color, q_feat, s_feat)
    if Mf not in _CACHE:
        _CACHE[Mf] = _build_program(Mf)
    nc = _CACHE[Mf]
    # slab: [128, 2*cols] with kc chunk at [kc*cols, (kc+1)*cols)
    q_slab = _bf16(np.concatenate([Qn[0:128, :], Qn[128:256, :]], axis=1))
    in_maps = []
    for c in range(NCORES):
        sc = sn[:, c * R:(c + 1) * R]
        s_slab = _bf16(np.concatenate([sc[0:128, :], sc[128:256, :]], axis=1))
        in_maps.append({"s16": s_slab, "q": q_slab})
    res = run_bass_kernel_spmd(nc, in_maps, list(range(NCORES)), trace=trace)
    fg = np.concatenate([res.results[c]["fg"].T.reshape(-1) for c in range(NCORES)])
    bg = np.concatenate([res.results[c]["bg"].T.reshape(-1) for c in range(NCORES)])
    return fg.reshape(H, W), bg.reshape(H, W), res


def kernel(query_label, color, q_feat, s_feat):
    fg, bg, _ = _run(query_label, color, q_feat, s_feat)
    return fg, bg


# revision 15
# speedup vs baseline: 1.1507x; 1.1507x over previous
"""Trainium2 Bass kernel for MatchingLayer (cosine-sim + per-row top-K mean).

Computation (reference):
  mask[m]  = all(query_label[m] == color)            # per-COLUMN property
  sim      = l2norm_rows(s) @ l2norm_rows(q).T       # [N=9216, M=9216], C=256
  fg_score = mean(top20(sim over fg columns)) per row -> (96, 96)
  bg_score = mean(top20(sim over bg columns)) per row -> (96, 96)

Sharding: rows split across 8 cores, 1152 rows each. Q replicated,
reordered fg-first; both s and q l2-normalized + bf16 on host.

Trace-driven evolution of the 137-163us baseline:
 * Slab input layout [128, 2*cols] (contraction-chunk concat per
   partition): long contiguous DMA lines instead of the 3168 small
   descriptors that trickled input in until the final microseconds and
   paced the whole kernel.
 * fg scoring via threshold-sum: exact top-8 per 192-wide span (6 max8),
   cascade -> exact top-24 of candidates, tau = 20th largest, then ACT
   computes sum(relu(x - tau)) over the fg columns in one instruction;
   fg = tau + S/20. Exact when the candidates cover the top-20,
   second-order-small error otherwise (sim/HW: 3.7e-3 relmax).
 * The next block's fg-tile matmuls are emitted after the 5th bg tile:
   by then this block's fg relu has released the fg banks (fgp bufs=1),
   so the PE fills the next fg tile mid-block and the DVE does not wait
   at the block boundary (~1.3us/block gap in the un-pipelined trace).
   Emitting them earlier (bufs=2 fgA variant) made every PSUM scan
   ~100ns slower from PE-write/DVE-read PSUM contention -- a net loss.
 * bg unchanged: exact top-8 per 1024 PSUM tile + cascade top-24 +
   mean(top20) via ACT accum (4.5e-3).

Per 128-row block: matmul 512-wide bf16 pieces into fg tile (3 banks,
bufs=1) + rolling 1024-col bg tiles (2x2 banks) + 512 tail bank. Every
sim value crosses DVE max8 at ~1 elem/cycle -- the architectural floor
(ACT cannot max, GPSIMD cannot read PSUM, matmul only writes PSUM).
"""

import sys

sys.path.insert(0, "/opt/trn_rl_repo")

import numpy as np

C = 256
H = W = 96
N = H * W            # 9216 support locations (rows of sim)
M = H * W            # 9216 query locations  (cols of sim)
NCORES = 8
R = N // NCORES      # 1152 rows per core
RB = R // 128        # 9 row blocks per core
K = 20
NEG = -1.0e30
FGW = 192            # fg candidate span width

_CACHE = {}


def _build_program(Mf):
    import concourse.mybir as mybir
    from concourse import bacc, tile

    f32 = mybir.dt.float32
    bf16 = mybir.dt.bfloat16

    nc = bacc.Bacc()
    s16_in = nc.declare_dram_parameter("s16", [128, 2 * R], bf16, isOutput=False)
    q_in = nc.declare_dram_parameter("q", [128, 2 * M], bf16, isOutput=False)
    fg_out = nc.declare_dram_parameter("fg", [128, RB], f32, isOutput=True)
    bg_out = nc.declare_dram_parameter("bg", [128, RB], f32, isOutput=True)

    assert 1024 < Mf <= 1528, f"unexpected fg column count {Mf}"

    # column tiling: fg tile [0,1536) = fg Mf + bg head;
    # then 7 bg tiles of 1024: [1536, 8704); tail [8704, 9216).
    BG1 = 1536
    NBG = 7
    TAIL = 8704
    assert BG1 + NBG * 1024 == TAIL and TAIL + 512 == M

    FG_SPANS = []
    b = 0
    while b < Mf:
        e = min(b + FGW, Mf)
        if 0 < Mf - e < 8:
            e = Mf
        FG_SPANS.append((b, e))
        b = e
    NFG = len(FG_SPANS)

    NBGL = 2 + NBG  # number of 8-wide bg candidate lists

    with tile.TileContext(nc) as tc:
        with (
            tc.tile_pool(name="const", bufs=1) as cp,
            tc.tile_pool(name="work", bufs=2) as wp,
            tc.tile_pool(name="fgp", bufs=1, space="PSUM") as fgp,
            tc.tile_pool(name="bgp", bufs=2, space="PSUM") as bgp,
            tc.tile_pool(name="tlp", bufs=1, space="PSUM") as tlp,
        ):
            Qs = cp.tile([128, 2 * M], bf16, tag="qs", name="qs")
            Ss = cp.tile([128, 2 * R], bf16, tag="ss", name="ss")
            out_fg = cp.tile([128, RB], f32, tag="out_fg")
            out_bg = cp.tile([128, RB], f32, tag="out_bg")

            # --- input DMAs: long contiguous per-partition lines ---
            nc.sync.dma_start(out=Ss[:], in_=s16_in[:])
            qranges = [(0, 512), (512, 1536), (1536, 5120), (5120, 9216)]
            for lo, hi in qranges:
                for kc in range(2):
                    nc.sync.dma_start(
                        out=Qs[:, kc * M + lo:kc * M + hi],
                        in_=q_in[:, kc * M + lo:kc * M + hi],
                    )

            MMW = 512  # matmul moving width (one PSUM bank per MM output;
                       # 1024-wide outputs fail neuronxcc codegen)

            def mm_fill(pt, psl, rb, csl):
                """Accumulate sim rows rb*128.. into pt[:, psl] for cols csl."""
                rlo = rb * 128
                width = psl.stop - psl.start
                assert width == csl.stop - csl.start
                pieces = []
                b = 0
                while b < width:
                    e = min(b + MMW, width)
                    pieces.append((b, e))
                    b = e
                for kc in range(2):
                    st = kc * R + rlo
                    qt = kc * M + csl.start
                    for b, e in pieces:
                        nc.tensor.matmul(
                            pt[:, psl.start + b:psl.start + e],
                            Ss[:, st:st + 128],
                            Qs[:, qt + b:qt + e],
                            start=(kc == 0), stop=(kc == 1),
                        )

            def emit_fg_tile(rb):
                t = fgp.tile([128, 1536], f32, tag="fgt")
                mm_fill(t, slice(0, 1024), rb, slice(0, 1024))
                mm_fill(t, slice(1024, 1536), rb, slice(1024, 1536))
                return t

            fgt_next = emit_fg_tile(0)

            # --- main loop: 9 row blocks ---
            for rb in range(RB):
                fgl = wp.tile([128, NFG * 8], f32, tag="fgl")
                fglb = wp.tile([128, NFG * 8], f32, tag="fglb")
                bgl = wp.tile([128, NBGL * 8], f32, tag="bgl")
                bglb = wp.tile([128, NBGL * 8], f32, tag="bglb")
                gf = wp.tile([128, 24], f32, tag="gf")
                gb = wp.tile([128, 24], f32, tag="gb")
                ntau = wp.tile([128, 1], f32, tag="ntau")
                facc = wp.tile([128, 1], f32, tag="facc")
                scr = wp.tile([128, K], f32, tag="scr")
                # relu spill target in SBUF: keeps ACT write traffic off
                # PSUM, which the DVE is scanning concurrently
                rscr = wp.tile([128, 1536], bf16, tag="rscr")

                # ---- fg tile scans ----
                fgt = fgt_next
                for i, (lo, hi) in enumerate(FG_SPANS):
                    nc.vector.max(fgl[:, i * 8:(i + 1) * 8], fgt[:, lo:hi])
                # bg span inside fg tile (cols Mf..1536)
                nc.vector.max(bgl[:, 0:8], fgt[:, Mf:1536])

                def fg_finish():
                    # fg cascade -> exact top-24 of candidates; emitted
                    # after B2's scan so the scheduler's encoded semaphore
                    # waits reference long-completed PE work (emitting it
                    # at block start left it sequenced behind the next
                    # fg-tile matmul batch -> ~1us/block DVE stall)
                    nc.vector.max(gf[:, 0:8], fgl[:])
                    nc.vector.match_replace(fglb[:], gf[:, 0:8], fgl[:], NEG)
                    nc.vector.max(gf[:, 8:16], fglb[:])
                    nc.vector.match_replace(fgl[:], gf[:, 8:16], fglb[:], NEG)
                    nc.vector.max(gf[:, 16:24], fgl[:])
                    # tau = 20th largest candidate; fg = tau + relu-sum/K
                    nc.vector.tensor_scalar_mul(ntau[:], gf[:, 19:20], -1.0)
                    nc.scalar.activation(
                        out=rscr[:, 0:Mf], in_=fgt[:, 0:Mf],
                        func=mybir.ActivationFunctionType.Relu,
                        bias=ntau[:, 0:1], scale=1.0,
                        accum_out=facc[:])
                    nc.vector.tensor_scalar(
                        out=out_fg[:, rb:rb + 1], in0=facc[:],
                        scalar1=1.0 / K, scalar2=gf[:, 19:20],
                        op0=mybir.AluOpType.mult, op1=mybir.AluOpType.add)

                # ---- bg sections ----
                # tail MMs first: its bank is free from the previous block,
                # giving the PE runway before the bgp bufs=2 stall
                tlt = tlp.tile([128, 512], f32, tag="tlt")
                mm_fill(tlt, slice(0, 512), rb, slice(TAIL, M))
                for j in range(NBG):
                    lo = BG1 + 1024 * j
                    bgt = bgp.tile([128, 1024], f32, tag="bgt")
                    mm_fill(bgt, slice(0, 1024), rb, slice(lo, lo + 1024))
                    nc.vector.max(bgl[:, (1 + j) * 8:(2 + j) * 8], bgt[:])
                    if j == 2:
                        fg_finish()
                    if j == 4 and rb + 1 < RB:
                        # next block's fg tile: the fg relu above has
                        # released the fgp banks by now, and the PE picks
                        # these up mid-block instead of at the boundary
                        fgt_next = emit_fg_tile(rb + 1)
                nc.vector.max(bgl[:, (1 + NBG) * 8:(2 + NBG) * 8], tlt[:])

                # ---- bg cascade -> exact top-24 + mean(top20) ----
                nc.vector.max(gb[:, 0:8], bgl[:])
                nc.vector.match_replace(bglb[:], gb[:, 0:8], bgl[:], NEG)
                nc.vector.max(gb[:, 8:16], bglb[:])
                nc.vector.match_replace(bgl[:], gb[:, 8:16], bglb[:], NEG)
                nc.vector.max(gb[:, 16:24], bgl[:])
                nc.scalar.activation(
                    out=scr[:, 0:K], in_=gb[:, 0:K],
                    func=mybir.ActivationFunctionType.Copy,
                    scale=1.0 / K, accum_out=out_bg[:, rb:rb + 1])

            nc.sync.dma_start(out=fg_out[:], in_=out_fg[:])
            nc.sync.dma_start(out=bg_out[:], in_=out_bg[:])

    nc.compile()
    return nc


def _bf16(a):
    import ml_dtypes
    return np.ascontiguousarray(a.astype(ml_dtypes.bfloat16))


def _prep_inputs(query_label, color, q_feat, s_feat):
    mask = np.all(np.asarray(query_label) == np.asarray(color), axis=-1).reshape(-1)
    Mf = int(mask.sum())
    q = np.asarray(q_feat, dtype=np.float32)[0].reshape(C, M)  # [C, M]
    s = np.asarray(s_feat, dtype=np.float32)[0].reshape(C, N)
    qn = q / np.maximum(np.sqrt(np.sum(q * q, axis=0)), np.float32(1e-12))[None, :]
    sn = s / np.maximum(np.sqrt(np.sum(s * s, axis=0)), np.float32(1e-12))[None, :]
    order = np.concatenate([np.nonzero(mask)[0], np.nonzero(~mask)[0]])
    Qn = np.ascontiguousarray(qn[:, order], dtype=np.float32)
    return Mf, Qn, sn


def _run(query_label, color, q_feat, s_feat, trace=False):
    from concourse.bass_utils import run_bass_kernel_spmd

    Mf, Qn, sn = _prep_inputs(query_label, color, q_feat, s_feat)
    if Mf not in _CACHE:
        _CACHE[Mf] = _build_program(Mf)
    nc = _CACHE[Mf]
    # slab: [128, 2*cols] with kc chunk at [kc*cols, (kc+1)*cols)
    q_slab = _bf16(np.concatenate([Qn[0:128, :], Qn[128:256, :]], axis=1))
    in_maps = []
    for c in range(NCORES):
        sc = sn[:, c * R:(c + 1) * R]
        s_slab = _bf16(np.concatenate([sc[0:128, :], sc[128:256, :]], axis=1))
        in_maps.append({"s16": s_slab, "q": q_slab})
    res = run_bass_kernel_spmd(nc, in_maps, list(range(NCORES)), trace=trace)
    fg = np.concatenate([res.results[c]["fg"].T.reshape(-1) for c in range(NCORES)])
    bg = np.concatenate([res.results[c]["bg"].T.reshape(-1) for c in range(NCORES)])
    return fg.reshape(H, W), bg.reshape(H, W), res


def kernel(query_label, color, q_feat, s_feat):
    fg, bg, _ = _run(query_label, color, q_feat, s_feat)
    return fg, bg
